# revision 17
# baseline (speedup 1.0000x reference)
"""Trainium2 Bass kernel for nn_MergeHeads (moe_routing).

Computes out[t] = sum_a p[t,a] * (x[t,a] @ W[idx[t,a]] + b[idx[t,a]])
for B*S = 16384 tokens, A=2 slots, H=8 heads, DH=128, DM=2048.

Strategy: data-parallel over tokens across 8 NeuronCores (2048 tokens
each); W/b replicated.  Per core, dense routed formulation computed
almost entirely on the TensorEngine in float32r (full PE rate at
N>=256, ~1.5e-4 max rel err):

  routing (per 128-token tile, as matmuls):
    G_a[t', (h,t)] = eye[t',t] * wgt[t',h,a]   (one DVE op per slot;
                                                wgt = (idx==h)*p)
    routedT[d, (h,t)] = sum_a x_a^T @ G_a      (4 accumulated matmuls)
  main (per tile):
    y[t, mc] = sum_h routedT[:,h,:].T @ W[h][:,mc] + wsT.T @ b[:,mc]
    (h-outer / mc-inner over 4 PSUM banks so each stationary loads once)

DMA layout: W as 8 contiguous 1MB loads split across both HWDGE rings;
x prefetched 3 tiles ahead on the sync ring; y stores issued from the
scalar ring (1MB per tile) so loads never queue behind stores.
"""

import os
import numpy as np

B, S, A, H, DH, DM = 4, 4096, 2, 8, 128, 2048
NCORES = 8
T = B * S
TLOC = T // NCORES        # 2048 tokens per core
P = 128                   # partitions / token tile
NT = TLOC // P            # 16 token tiles per core
NFREE = 512               # matmul moving free dim (one PSUM bank of fp32)
MC = DM // NFREE          # 4 output chunks per token tile
PREF = 2                  # x-tile prefetch distance (ahead of routing)
DA = 3                    # routing runs this many tiles ahead of phase-A mains

# compute dtype: "f32r" (default), "bf16", or "f32"
CDT_MODE = os.environ.get("TRNK_DTYPE", "f32r")

_CACHE = {}


def _build_nc():
    import concourse.mybir as mybir
    from concourse import bacc
    from concourse.tile import TileContext
    from concourse.masks import make_identity

    f32 = mybir.dt.float32
    cdt = {
        "f32r": mybir.dt.float32r,
        "bf16": mybir.dt.bfloat16,
        "f32": mybir.dt.float32,
    }[CDT_MODE]

    nc = bacc.Bacc("TRN2", target_bir_lowering=False, debug=False)

    x_d = nc.dram_tensor("x", [TLOC, A, DH], f32, kind="ExternalInput")
    idx_d = nc.dram_tensor("idxf", [TLOC, A], f32, kind="ExternalInput")
    p_d = nc.dram_tensor("p", [TLOC, A], f32, kind="ExternalInput")
    w_d = nc.dram_tensor("W", [H, DH, DM], f32, kind="ExternalInput")
    b_d = nc.dram_tensor("b", [H, DM], f32, kind="ExternalInput")
    hg_d = nc.dram_tensor("hgrid", [P, H, A], f32, kind="ExternalInput")
    y_d = nc.dram_tensor("out", [TLOC, DM], f32, kind="ExternalOutput")

    with TileContext(nc) as tc:
        with tc.tile_pool(name="const", bufs=1) as const, \
             tc.tile_pool(name="wstage", bufs=2) as wstage, \
             tc.tile_pool(name="xpool", bufs=PREF + 1) as xpool, \
             tc.tile_pool(name="xrpool", bufs=4) as xrpool, \
             tc.tile_pool(name="gpool", bufs=5) as gpool, \
             tc.tile_pool(name="rpool", bufs=NT) as rpool, \
             tc.tile_pool(name="wst", bufs=NT) as wstpool, \
             tc.tile_pool(name="ypool", bufs=3) as ypool, \
             tc.tile_pool(name="pr", bufs=3, space="PSUM") as prpool, \
             tc.tile_pool(name="py", bufs=4, space="PSUM") as pypool, \
             tc.tile_pool(name="pw", bufs=1, space="PSUM") as pwpool:

            # ---- constants / setup ----
            eye = const.tile([P, P], f32, tag="eye")
            make_identity(nc, eye[:])

            hg = const.tile([P, H, A], f32, tag="hg")
            nc.sync.dma_start(hg[:], hg_d[:])

            idx_sb = const.tile([P, NT, A], f32, tag="idx")
            p_sb = const.tile([P, NT, A], f32, tag="p")
            # dst[tp, i, a] = src[i*P + tp, a]
            nc.sync.dma_start(idx_sb[:], idx_d[:].rearrange("(i tp) a -> tp i a", tp=P))
            nc.sync.dma_start(p_sb[:], p_d[:].rearrange("(i tp) a -> tp i a", tp=P))

            # W -> SBUF, rounded: W_r[d, h, m].  Loaded half-major
            # (columns 0:1024 for every head first) so phase-A mains can
            # start as soon as the first half lands; split across both
            # HWDGE rings.
            HDM = DM // 2
            w_r = const.tile([P, H, DM], cdt, tag="w_r")
            for half in range(2):
                for h in range(H):
                    st = wstage.tile([P, HDM], f32, tag="wst")
                    ring = nc.sync if h % 2 == 0 else nc.scalar
                    ring.dma_start(st[:], w_d[h, :, half * HDM:(half + 1) * HDM])
                    dst = w_r[:, h, half * HDM:(half + 1) * HDM]
                    if h % 2 == 0:
                        nc.vector.tensor_copy(dst, st[:])
                    else:
                        nc.scalar.copy(dst, st[:])

            # b -> SBUF rounded: b_r[h, m] (partitions 0..7); stage via
            # the wstage pool slot to save SBUF.
            bstage = wstage.tile([P, DM], f32, tag="wst")
            nc.sync.dma_start(bstage[:H, :], b_d[:])
            b_r = const.tile([H, DM], cdt, tag="b_r")
            nc.vector.tensor_copy(b_r[:], bstage[:H, :])

            # routing weights wgt[tp, i, h, a] = (idx==h) * p
            wgt = const.tile([P, NT, H, A], f32, tag="wgt")
            idx_b = idx_sb[:].unsqueeze(2).broadcast_to([P, NT, H, A])
            p_b = p_sb[:].unsqueeze(2).broadcast_to([P, NT, H, A])
            hg_b = hg[:].unsqueeze(1).broadcast_to([P, NT, H, A])
            nc.vector.tensor_tensor(wgt[:], idx_b, hg_b, mybir.AluOpType.is_equal)
            nc.vector.tensor_tensor(wgt[:], wgt[:], p_b, mybir.AluOpType.mult)
            # per-head prob sums wsum[tp, i, h] = wgt[...,0] + wgt[...,1]
            wsum = const.tile([P, NT, H], f32, tag="wsum")
            nc.vector.tensor_tensor(
                wsum[:], wgt[:, :, :, 0], wgt[:, :, :, 1], mybir.AluOpType.add
            )

            eye_b = eye[:].unsqueeze(1).broadcast_to([P, H, P])

            # x tile prefetch
            x_tiles = {}

            def issue_x(i):
                if i < NT and i not in x_tiles:
                    x_t = xpool.tile([P, A, DH], f32, tag="x")
                    nc.sync.dma_start(x_t[:], x_d[i * P:(i + 1) * P, :, :])
                    x_tiles[i] = x_t

            for i in range(PREF):
                issue_x(i)

            # ---- main pipeline: routing runs DEPTH tiles ahead ----
            # prep (x_r round + G builds) is emitted GPREF tiles ahead of
            # the routing matmuls so the per-engine in-order streams never
            # couple next-tile producers to previous-tile main consumers.
            GPREF = 2
            prepped = {}  # tile idx -> (x_r, g0, g1)

            def prep(i):
                if i >= NT or i in prepped:
                    return
                x_t = x_tiles.pop(i)
                x_r = xrpool.tile([P, A, DH], cdt, tag="xr")
                nc.scalar.copy(x_r[:], x_t[:])
                # G_a[t', (h,t)] = eye * wgt (rounded on write); split
                # DVE/ACT.  GpSimd must stay idle (it contends with DVE
                # for the shared SBUF port pair).
                g0 = gpool.tile([P, H, P], cdt, tag="g")
                g1 = gpool.tile([P, H, P], cdt, tag="g")
                w0_b = wgt[:, i, :, 0].unsqueeze(2).broadcast_to([P, H, P])
                w1_b = wgt[:, i, :, 1].unsqueeze(2).broadcast_to([P, H, P])
                nc.vector.tensor_tensor(g0[:], eye_b, w0_b, mybir.AluOpType.mult)
                nc.vector.tensor_tensor(
                    g1[:, 0:6, :], eye_b[:, 0:6, :], w1_b[:, 0:6, :],
                    mybir.AluOpType.mult)
                for h in range(6, H):
                    nc.scalar.activation(
                        g1[:, h, :], eye[:],
                        mybir.ActivationFunctionType.Copy,
                        scale=wgt[:, i, h, 1:2])
                prepped[i] = (x_r, g0, g1)

            pending = {}  # tile idx -> (routedT tile, wsT tile)

            def main_half(j, half):
                # 2 output chunks (one PSUM bank each): 8 routed heads
                # accumulated + bias, h-outer so each stationary streams
                # both chunks.
                r_p, ws_p = pending[j]
                mcs = (2 * half, 2 * half + 1)
                py_ts = {}
                for mc in mcs:
                    py_t = pypool.tile([P, NFREE], f32, tag="py")
                    py_ts[mc] = py_t
                for h in range(H):
                    for mc in mcs:
                        nc.tensor.matmul(
                            py_ts[mc][:],
                            r_p[:, h, :],
                            w_r[:, h, mc * NFREE:(mc + 1) * NFREE],
                            start=(h == 0), stop=False,
                        )
                for mc in mcs:
                    nc.tensor.matmul(
                        py_ts[mc][:], ws_p[:],
                        b_r[:, mc * NFREE:(mc + 1) * NFREE],
                        start=False, stop=True,
                    )
                y_t = ypool.tile([P, DM // 2], f32, tag="y")
                nc.vector.tensor_copy(y_t[:, 0:NFREE], py_ts[mcs[0]][:])
                nc.scalar.copy(y_t[:, NFREE:2 * NFREE], py_ts[mcs[1]][:])
                # phase A stores ride the scalar ring (sync ring is busy
                # with x/W loads); phase B uses the then-idle sync ring.
                ring = nc.scalar if half == 0 else nc.sync
                ring.dma_start(
                    y_d[j * P:(j + 1) * P, half * HDM:(half + 1) * HDM],
                    y_t[:],
                )

            for i in range(NT + DA):
                issue_x(i + PREF)
                prep(i + GPREF)
                if i < NT:
                    prep(i)
                    x_r, g0, g1 = prepped.pop(i)

                    # routedT[d, (h,t)] = sum_a x_a^T @ G_a  (2 PSUM banks)
                    r_t = rpool.tile([P, H, DH], cdt, tag="r")
                    pr0 = prpool.tile([P, NFREE], f32, tag="pr")
                    pr1 = prpool.tile([P, NFREE], f32, tag="pr")
                    g0f = g0[:].rearrange("p h t -> p (h t)")
                    g1f = g1[:].rearrange("p h t -> p (h t)")
                    nc.tensor.matmul(pr0[:], x_r[:, 0, :], g0f[:, 0:NFREE],
                                     start=True, stop=False)
                    nc.tensor.matmul(pr1[:], x_r[:, 0, :], g0f[:, NFREE:2 * NFREE],
                                     start=True, stop=False)
                    nc.tensor.matmul(pr0[:], x_r[:, 1, :], g1f[:, 0:NFREE],
                                     start=False, stop=True)
                    nc.tensor.matmul(pr1[:], x_r[:, 1, :], g1f[:, NFREE:2 * NFREE],
                                     start=False, stop=True)
                    dst0 = r_t[:, 0:4, :].rearrange("p a b -> p (a b)")
                    dst1 = r_t[:, 4:8, :].rearrange("p a b -> p (a b)")
                    nc.vector.tensor_copy(dst0, pr0[:])
                    nc.scalar.copy(dst1, pr1[:])

                    # transposed per-head prob sums for the bias matmul
                    pw_t = pwpool.tile([H, P], f32, tag="pw")
                    nc.tensor.transpose(pw_t[:], wsum[:, i, :], eye[:])
                    ws_t = wstpool.tile([H, P], cdt, tag="ws")
                    nc.scalar.copy(ws_t[:], pw_t[:])
                    pending[i] = (r_t, ws_t)

                if i >= DA:
                    main_half(i - DA, 0)

            # phase B: pure matmul stream over the second output half
            for j in range(NT):
                main_half(j, 1)
                pending.pop(j)

    nc.compile()
    return nc


def _get_nc():
    if "nc" not in _CACHE:
        _CACHE["nc"] = _build_nc()
    return _CACHE["nc"]


def kernel(embedding, sel_idx, sel_probs, W, b):
    from concourse.bass_utils import run_bass_kernel_spmd

    emb = np.ascontiguousarray(embedding, dtype=np.float32).reshape(T, A, DH)
    idxf = np.ascontiguousarray(sel_idx).reshape(T, A).astype(np.float32)
    pf = np.ascontiguousarray(sel_probs, dtype=np.float32).reshape(T, A)
    Wf = np.ascontiguousarray(W, dtype=np.float32)
    bf = np.ascontiguousarray(b, dtype=np.float32)
    hgrid = np.ascontiguousarray(
        np.broadcast_to(
            np.arange(H, dtype=np.float32)[None, :, None], (P, H, A)
        )
    )

    nc = _get_nc()
    in_maps = []
    for c in range(NCORES):
        sl = slice(c * TLOC, (c + 1) * TLOC)
        in_maps.append({
            "x": emb[sl],
            "idxf": idxf[sl],
            "p": pf[sl],
            "W": Wf,
            "b": bf,
            "hgrid": hgrid,
        })

    trace = os.environ.get("TRNK_TRACE") == "1"
    if trace:
        _register_ntff_stub()
    res = run_bass_kernel_spmd(
        nc, in_maps, core_ids=list(range(NCORES)), trace=trace
    )
    if trace:
        _CACHE["exec_time_ns"] = res.exec_time_ns
        _CACHE["results_obj"] = res

    out = np.concatenate(
        [res.results[c]["out"] for c in range(NCORES)], axis=0
    )
    return out.reshape(B, S, DM)


def _register_ntff_stub():
    """antenv.axon_hooks is absent in this image; back it with the boot
    ctypes NTFF hook so trace=True works under axon."""
    import sys, types
    try:
        import antenv.axon_hooks  # noqa: F401
        return
    except ImportError:
        pass
    try:
        import antenv
        from trn_agent_boot.trn_boot import _ntff_profile_via_ctypes
    except ImportError:
        return
    mod = types.ModuleType("antenv.axon_hooks")
    hook = [None]

    def set_axon_ntff_profile_hook(h):
        hook[0] = h

    def get_axon_ntff_profile_hook():
        if hook[0] is None:
            hook[0] = _ntff_profile_via_ctypes("/opt/axon/libaxon_pjrt.so")
        return hook[0]

    mod.set_axon_ntff_profile_hook = set_axon_ntff_profile_hook
    mod.get_axon_ntff_profile_hook = get_axon_ntff_profile_hook
    sys.modules["antenv.axon_hooks"] = mod
    antenv.axon_hooks = mod


# revision 18
# speedup vs baseline: 1.0142x; 1.0142x over previous
"""Trainium2 Bass kernel for nn_MergeHeads (moe_routing).

Computes out[t] = sum_a p[t,a] * (x[t,a] @ W[idx[t,a]] + b[idx[t,a]])
for B*S = 16384 tokens, A=2 slots, H=8 heads, DH=128, DM=2048.

Strategy: data-parallel over tokens across 8 NeuronCores (2048 tokens
each); W/b replicated.  Per core, dense routed formulation computed
almost entirely on the TensorEngine in float32r (full PE rate at
N>=256, ~1.5e-4 max rel err):

  routing (per 128-token tile, as matmuls):
    G_a[t', (h,t)] = eye[t',t] * wgt[t',h,a]   (one DVE op per slot;
                                                wgt = (idx==h)*p)
    routedT[d, (h,t)] = sum_a x_a^T @ G_a      (4 accumulated matmuls)
  main (per tile):
    y[t, mc] = sum_h routedT[:,h,:].T @ W[h][:,mc] + wsT.T @ b[:,mc]
    (h-outer / mc-inner over 4 PSUM banks so each stationary loads once)

DMA layout: W as 8 contiguous 1MB loads split across both HWDGE rings;
x prefetched 3 tiles ahead on the sync ring; y stores issued from the
scalar ring (1MB per tile) so loads never queue behind stores.
"""

import os
import numpy as np

B, S, A, H, DH, DM = 4, 4096, 2, 8, 128, 2048
NCORES = 8
T = B * S
TLOC = T // NCORES        # 2048 tokens per core
P = 128                   # partitions / token tile
NT = TLOC // P            # 16 token tiles per core
NFREE = 512               # matmul moving free dim (one PSUM bank of fp32)
MC = DM // NFREE          # 4 output chunks per token tile
PREF = 2                  # x-tile prefetch distance (ahead of routing)
DA = 3                    # routing runs this many tiles ahead of phase-A mains

# compute dtype: "f32r" (default), "bf16", or "f32"
CDT_MODE = os.environ.get("TRNK_DTYPE", "f32r")

_CACHE = {}


def _build_nc():
    import concourse.mybir as mybir
    from concourse import bacc
    from concourse.tile import TileContext
    from concourse.masks import make_identity

    f32 = mybir.dt.float32
    cdt = {
        "f32r": mybir.dt.float32r,
        "bf16": mybir.dt.bfloat16,
        "f32": mybir.dt.float32,
    }[CDT_MODE]

    nc = bacc.Bacc("TRN2", target_bir_lowering=False, debug=False)

    x_d = nc.dram_tensor("x", [TLOC, A, DH], f32, kind="ExternalInput")
    idx_d = nc.dram_tensor("idxf", [TLOC, A], f32, kind="ExternalInput")
    p_d = nc.dram_tensor("p", [TLOC, A], f32, kind="ExternalInput")
    w_d = nc.dram_tensor("W", [H, DH, DM], f32, kind="ExternalInput")
    b_d = nc.dram_tensor("b", [H, DM], f32, kind="ExternalInput")
    hg_d = nc.dram_tensor("hgrid", [P, H, A], f32, kind="ExternalInput")
    y_d = nc.dram_tensor("out", [TLOC, DM], f32, kind="ExternalOutput")

    with TileContext(nc) as tc:
        with tc.tile_pool(name="const", bufs=1) as const, \
             tc.tile_pool(name="wstage", bufs=2) as wstage, \
             tc.tile_pool(name="xpool", bufs=PREF + 1) as xpool, \
             tc.tile_pool(name="xrpool", bufs=4) as xrpool, \
             tc.tile_pool(name="gpool", bufs=5) as gpool, \
             tc.tile_pool(name="rpool", bufs=5) as rpool, \
             tc.tile_pool(name="wst", bufs=5) as wstpool, \
             tc.tile_pool(name="ypool", bufs=3) as ypool, \
             tc.tile_pool(name="pr", bufs=3, space="PSUM") as prpool, \
             tc.tile_pool(name="py", bufs=4, space="PSUM") as pypool, \
             tc.tile_pool(name="pw", bufs=1, space="PSUM") as pwpool:

            # ---- constants / setup ----
            eye = const.tile([P, P], f32, tag="eye")
            make_identity(nc, eye[:])

            hg = const.tile([P, H, A], f32, tag="hg")
            nc.sync.dma_start(hg[:], hg_d[:])

            idx_sb = const.tile([P, NT, A], f32, tag="idx")
            p_sb = const.tile([P, NT, A], f32, tag="p")
            # dst[tp, i, a] = src[i*P + tp, a]
            nc.sync.dma_start(idx_sb[:], idx_d[:].rearrange("(i tp) a -> tp i a", tp=P))
            nc.sync.dma_start(p_sb[:], p_d[:].rearrange("(i tp) a -> tp i a", tp=P))

            # W -> SBUF, rounded: W_r[d, h, m].  Loaded half-major
            # (columns 0:1024 for every head first) so phase-A mains can
            # start as soon as the first half lands; split across both
            # HWDGE rings.
            HDM = DM // 2
            w_r = const.tile([P, H, DM], cdt, tag="w_r")
            for half in range(2):
                for h in range(H):
                    st = wstage.tile([P, HDM], f32, tag="wst")
                    ring = nc.sync if h % 2 == 0 else nc.scalar
                    ring.dma_start(st[:], w_d[h, :, half * HDM:(half + 1) * HDM])
                    dst = w_r[:, h, half * HDM:(half + 1) * HDM]
                    if h % 2 == 0:
                        nc.vector.tensor_copy(dst, st[:])
                    else:
                        nc.scalar.copy(dst, st[:])

            # b -> SBUF rounded: b_r[h, m] (partitions 0..7); stage via
            # the wstage pool slot to save SBUF.
            bstage = wstage.tile([P, DM], f32, tag="wst")
            nc.sync.dma_start(bstage[:H, :], b_d[:])
            b_r = const.tile([H, DM], cdt, tag="b_r")
            nc.vector.tensor_copy(b_r[:], bstage[:H, :])

            # routing weights wgt[tp, i, h, a] = (idx==h) * p
            wgt = const.tile([P, NT, H, A], f32, tag="wgt")
            idx_b = idx_sb[:].unsqueeze(2).broadcast_to([P, NT, H, A])
            p_b = p_sb[:].unsqueeze(2).broadcast_to([P, NT, H, A])
            hg_b = hg[:].unsqueeze(1).broadcast_to([P, NT, H, A])
            nc.vector.tensor_tensor(wgt[:], idx_b, hg_b, mybir.AluOpType.is_equal)
            nc.vector.tensor_tensor(wgt[:], wgt[:], p_b, mybir.AluOpType.mult)
            # per-head prob sums wsum[tp, i, h] = wgt[...,0] + wgt[...,1]
            wsum = const.tile([P, NT, H], f32, tag="wsum")
            nc.vector.tensor_tensor(
                wsum[:], wgt[:, :, :, 0], wgt[:, :, :, 1], mybir.AluOpType.add
            )

            eye_b = eye[:].unsqueeze(1).broadcast_to([P, H, P])

            # x tile prefetch
            x_tiles = {}

            def issue_x(i):
                if i < NT and i not in x_tiles:
                    x_t = xpool.tile([P, A, DH], f32, tag="x")
                    nc.sync.dma_start(x_t[:], x_d[i * P:(i + 1) * P, :, :])
                    x_tiles[i] = x_t

            for i in range(PREF):
                issue_x(i)

            # ---- main pipeline: routing runs DEPTH tiles ahead ----
            # prep (x_r round + G builds) is emitted GPREF tiles ahead of
            # the routing matmuls so the per-engine in-order streams never
            # couple next-tile producers to previous-tile main consumers.
            GPREF = 2
            prepped = {}  # tile idx -> (x_r, g0, g1)

            def prep(i):
                if i >= NT or i in prepped:
                    return
                x_t = x_tiles.pop(i)
                x_r = xrpool.tile([P, A, DH], cdt, tag="xr")
                nc.scalar.copy(x_r[:], x_t[:])
                # G_a[t', (h,t)] = eye * wgt (rounded on write); split
                # DVE/ACT.  GpSimd must stay idle (it contends with DVE
                # for the shared SBUF port pair).
                g0 = gpool.tile([P, H, P], cdt, tag="g")
                g1 = gpool.tile([P, H, P], cdt, tag="g")
                w0_b = wgt[:, i, :, 0].unsqueeze(2).broadcast_to([P, H, P])
                w1_b = wgt[:, i, :, 1].unsqueeze(2).broadcast_to([P, H, P])
                nc.vector.tensor_tensor(g0[:], eye_b, w0_b, mybir.AluOpType.mult)
                nc.vector.tensor_tensor(
                    g1[:, 0:6, :], eye_b[:, 0:6, :], w1_b[:, 0:6, :],
                    mybir.AluOpType.mult)
                for h in range(6, H):
                    nc.scalar.activation(
                        g1[:, h, :], eye[:],
                        mybir.ActivationFunctionType.Copy,
                        scale=wgt[:, i, h, 1:2])
                prepped[i] = (x_r, g0, g1)

            pending = {}  # tile idx -> (routedT tile, wsT tile)

            def main_half(j, half):
                # 2 output chunks (one PSUM bank each): 8 routed heads
                # accumulated + bias, h-outer so each stationary streams
                # both chunks.
                r_p, ws_p = pending[j]
                mcs = tuple(range(MC)) if half == 0 else ()
                py_ts = {}
                for mc in mcs:
                    py_t = pypool.tile([P, NFREE], f32, tag="py")
                    py_ts[mc] = py_t
                for h in range(H):
                    for mc in mcs:
                        nc.tensor.matmul(
                            py_ts[mc][:],
                            r_p[:, h, :],
                            w_r[:, h, mc * NFREE:(mc + 1) * NFREE],
                            start=(h == 0), stop=False,
                        )
                for mc in mcs:
                    nc.tensor.matmul(
                        py_ts[mc][:], ws_p[:],
                        b_r[:, mc * NFREE:(mc + 1) * NFREE],
                        start=False, stop=True,
                    )
                y_t = ypool.tile([P, DM], f32, tag="y")
                for mc in mcs:
                    dst = y_t[:, mc * NFREE:(mc + 1) * NFREE]
                    if mc % 2 == 0:
                        nc.vector.tensor_copy(dst, py_ts[mc][:])
                    else:
                        nc.scalar.copy(dst, py_ts[mc][:])
                nc.scalar.dma_start(y_d[j * P:(j + 1) * P, :], y_t[:])

            for i in range(NT + DA):
                issue_x(i + PREF)
                prep(i + GPREF)
                if i < NT:
                    prep(i)
                    x_r, g0, g1 = prepped.pop(i)

                    # routedT[d, (h,t)] = sum_a x_a^T @ G_a  (2 PSUM banks)
                    r_t = rpool.tile([P, H, DH], cdt, tag="r")
                    pr0 = prpool.tile([P, NFREE], f32, tag="pr")
                    pr1 = prpool.tile([P, NFREE], f32, tag="pr")
                    g0f = g0[:].rearrange("p h t -> p (h t)")
                    g1f = g1[:].rearrange("p h t -> p (h t)")
                    nc.tensor.matmul(pr0[:], x_r[:, 0, :], g0f[:, 0:NFREE],
                                     start=True, stop=False)
                    nc.tensor.matmul(pr1[:], x_r[:, 0, :], g0f[:, NFREE:2 * NFREE],
                                     start=True, stop=False)
                    nc.tensor.matmul(pr0[:], x_r[:, 1, :], g1f[:, 0:NFREE],
                                     start=False, stop=True)
                    nc.tensor.matmul(pr1[:], x_r[:, 1, :], g1f[:, NFREE:2 * NFREE],
                                     start=False, stop=True)
                    dst0 = r_t[:, 0:4, :].rearrange("p a b -> p (a b)")
                    dst1 = r_t[:, 4:8, :].rearrange("p a b -> p (a b)")
                    nc.vector.tensor_copy(dst0, pr0[:])
                    nc.scalar.copy(dst1, pr1[:])

                    # transposed per-head prob sums for the bias matmul
                    pw_t = pwpool.tile([H, P], f32, tag="pw")
                    nc.tensor.transpose(pw_t[:], wsum[:, i, :], eye[:])
                    ws_t = wstpool.tile([H, P], cdt, tag="ws")
                    nc.scalar.copy(ws_t[:], pw_t[:])
                    pending[i] = (r_t, ws_t)

                if i >= DA:
                    main_half(i - DA, 0)
                    pending.pop(i - DA)



    nc.compile()
    return nc


def _get_nc():
    if "nc" not in _CACHE:
        _CACHE["nc"] = _build_nc()
    return _CACHE["nc"]


def kernel(embedding, sel_idx, sel_probs, W, b):
    from concourse.bass_utils import run_bass_kernel_spmd

    emb = np.ascontiguousarray(embedding, dtype=np.float32).reshape(T, A, DH)
    idxf = np.ascontiguousarray(sel_idx).reshape(T, A).astype(np.float32)
    pf = np.ascontiguousarray(sel_probs, dtype=np.float32).reshape(T, A)
    Wf = np.ascontiguousarray(W, dtype=np.float32)
    bf = np.ascontiguousarray(b, dtype=np.float32)
    hgrid = np.ascontiguousarray(
        np.broadcast_to(
            np.arange(H, dtype=np.float32)[None, :, None], (P, H, A)
        )
    )

    nc = _get_nc()
    in_maps = []
    for c in range(NCORES):
        sl = slice(c * TLOC, (c + 1) * TLOC)
        in_maps.append({
            "x": emb[sl],
            "idxf": idxf[sl],
            "p": pf[sl],
            "W": Wf,
            "b": bf,
            "hgrid": hgrid,
        })

    trace = os.environ.get("TRNK_TRACE") == "1"
    if trace:
        _register_ntff_stub()
    res = run_bass_kernel_spmd(
        nc, in_maps, core_ids=list(range(NCORES)), trace=trace
    )
    if trace:
        _CACHE["exec_time_ns"] = res.exec_time_ns
        _CACHE["results_obj"] = res

    out = np.concatenate(
        [res.results[c]["out"] for c in range(NCORES)], axis=0
    )
    return out.reshape(B, S, DM)


def _register_ntff_stub():
    """antenv.axon_hooks is absent in this image; back it with the boot
    ctypes NTFF hook so trace=True works under axon."""
    import sys, types
    try:
        import antenv.axon_hooks  # noqa: F401
        return
    except ImportError:
        pass
    try:
        import antenv
        from trn_agent_boot.trn_boot import _ntff_profile_via_ctypes
    except ImportError:
        return
    mod = types.ModuleType("antenv.axon_hooks")
    hook = [None]

    def set_axon_ntff_profile_hook(h):
        hook[0] = h

    def get_axon_ntff_profile_hook():
        if hook[0] is None:
            hook[0] = _ntff_profile_via_ctypes("/opt/axon/libaxon_pjrt.so")
        return hook[0]

    mod.set_axon_ntff_profile_hook = set_axon_ntff_profile_hook
    mod.get_axon_ntff_profile_hook = get_axon_ntff_profile_hook
    sys.modules["antenv.axon_hooks"] = mod
    antenv.axon_hooks = mod
